# revision 1
# baseline (speedup 1.0000x reference)
"""Trainium2 Bass kernel for nn_AttentionBlock (B=16, C=256, H=W=32, NH=4, GROUPS=8).

Strategy: data-parallel over batch. 8 cores x 2 batch elements each; no
collectives. Per batch element, everything is kept in [channels, spatial]
layout (channels on SBUF partitions):

  1. GroupNorm: per-channel sum/sumsq on DVE (free-dim reduce), group
     aggregation via a tiny matmul against a block-diagonal averaging matrix,
     rstd = exp(-0.5*ln(var+eps)) on ACT (stays inside the
     natural_log_exp_and_others table set used by softmax's exp).
  2. qkv 1x1 conv: Q,K produced as [o, s] tiles (weights stationary);
     V produced directly TRANSPOSED as v^T [s, d-block] tiles (hn stationary)
     so the attention PV matmul needs no on-chip transpose.
  3. Attention per head (d=64): scores computed transposed,
     S^T[k, q] = K_dS^T . Q_dS, with two heads packed into the PE array via
     64x128 row tiling. exp on ACT reads PSUM directly, writes bf16 expS^T
     to SBUF (scale=1/8 folded into the activation). P@V computed as
     out^T[d, q] = V^T_lhsT . expS^T[k, q] with two heads via 128x64 column
     tiling; softmax denominators via an extra matmul with a [1|0...]
     stationary block (row 0 = sum over k), same 128x64 mode.
  4. Normalize with DVE reciprocal + gpsimd partition-broadcast + DVE mult.
  5. proj 1x1 conv + residual + bias fused into the PSUM->SBUF evacuation.

Matmuls run in bf16 (1 cycle/row on TRN2 PE); GroupNorm statistics stay fp32.
"""

import sys

sys.path.insert(0, "/opt/trn_rl_repo")

from contextlib import ExitStack

import numpy as np
import ml_dtypes

import concourse.bass as bass
import concourse.tile as tile
from concourse import bacc, mybir
from concourse.bass_utils import run_bass_kernel_spmd

F32 = mybir.dt.float32
BF16 = mybir.dt.bfloat16
AF = mybir.ActivationFunctionType
OP = mybir.AluOpType

N_CORES = 8
B_PER = 2          # batch elements per core
C = 256
S = 1024           # H*W
NH = 4
D = 64             # head dim
EPS = 1e-5
CT = C // 128      # channel tiles (2)
KT = S // 128      # key/s tiles (8)
QC = S // 512      # q chunks of 512 (2)


def build_nc():
    nc = bacc.Bacc("TRN2", target_bir_lowering=False, debug=False,
                   num_devices=N_CORES)

    x_d = nc.dram_tensor("x", [B_PER, C, S], F32, kind="ExternalInput").ap()
    wqkvT_d = nc.dram_tensor("wqkvT", [C, 3 * C], BF16, kind="ExternalInput").ap()
    wprojT_d = nc.dram_tensor("wprojT", [C, C], BF16, kind="ExternalInput").ap()
    qkb_d = nc.dram_tensor("qkb", [128, 4], F32, kind="ExternalInput").ap()
    bv_d = nc.dram_tensor("bv", [128, C], F32, kind="ExternalInput").ap()
    pb_d = nc.dram_tensor("pb", [128, 2], F32, kind="ExternalInput").ap()
    nw_d = nc.dram_tensor("nw", [128, 2], F32, kind="ExternalInput").ap()
    nb_d = nc.dram_tensor("nb", [128, 2], F32, kind="ExternalInput").ap()
    g_d = nc.dram_tensor("G", [128, 128], F32, kind="ExternalInput").ap()
    dw_d = nc.dram_tensor("denw", [128, 64], BF16, kind="ExternalInput").ap()
    out_d = nc.dram_tensor("out", [B_PER, C, S], F32, kind="ExternalOutput").ap()

    with tile.TileContext(nc) as tc, ExitStack() as ctx:
        # ---- pools (bufs is per-tag) ----
        cpool = ctx.enter_context(tc.tile_pool(name="consts", bufs=1))
        xpool = ctx.enter_context(tc.tile_pool(name="x", bufs=1))
        hnpool = ctx.enter_context(tc.tile_pool(name="hn", bufs=1))
        qkpool = ctx.enter_context(tc.tile_pool(name="qk", bufs=1))
        vtpool = ctx.enter_context(tc.tile_pool(name="vt", bufs=1))
        expool = ctx.enter_context(tc.tile_pool(name="expS", bufs=1))
        atpool = ctx.enter_context(tc.tile_pool(name="attn", bufs=1))
        upool = ctx.enter_context(tc.tile_pool(name="u", bufs=2))
        bcpool = ctx.enter_context(tc.tile_pool(name="bcast", bufs=2))
        opool = ctx.enter_context(tc.tile_pool(name="osb", bufs=2))
        scpool = ctx.enter_context(tc.tile_pool(name="scratch", bufs=1))
        vecpool = ctx.enter_context(tc.tile_pool(name="vec", bufs=2))

        ps_scores = ctx.enter_context(tc.tile_pool(name="ps_sc", bufs=2,
                                                   space="PSUM"))
        ps_attn = ctx.enter_context(tc.tile_pool(name="ps_at", bufs=1,
                                                 space="PSUM"))
        ps_qkv = ctx.enter_context(tc.tile_pool(name="ps_qkv", bufs=2,
                                                space="PSUM"))

        # ---- constants ----
        wq = [cpool.tile([128, 3 * C], BF16, name=f"wq{i}", tag=f"wq{i}")
              for i in range(CT)]
        for i in range(CT):
            nc.sync.dma_start(wq[i][:], wqkvT_d[128 * i:128 * (i + 1), :])
        wp = [cpool.tile([128, C], BF16, name=f"wp{i}", tag=f"wp{i}")
              for i in range(CT)]
        for i in range(CT):
            nc.sync.dma_start(wp[i][:], wprojT_d[128 * i:128 * (i + 1), :])
        qkb = cpool.tile([128, 4], F32, name="qkb", tag="qkb")
        nc.sync.dma_start(qkb[:], qkb_d[:])
        bv = cpool.tile([128, C], F32, name="bv", tag="bv")
        nc.sync.dma_start(bv[:], bv_d[:])
        pb = cpool.tile([128, 2], F32, name="pb", tag="pb")
        nc.sync.dma_start(pb[:], pb_d[:])
        nw = cpool.tile([128, 2], F32, name="nw", tag="nw")
        nc.sync.dma_start(nw[:], nw_d[:])
        nb = cpool.tile([128, 2], F32, name="nb", tag="nb")
        nc.sync.dma_start(nb[:], nb_d[:])
        G = cpool.tile([128, 128], F32, name="G", tag="G")
        nc.sync.dma_start(G[:], g_d[:])
        denw = cpool.tile([128, 64], BF16, name="denw", tag="denw")
        nc.sync.dma_start(denw[:], dw_d[:])
        epsb = cpool.tile([128, 1], F32, name="epsb", tag="epsb")
        nc.vector.memset(epsb[:], EPS)

        # per-batch state
        xt = {}      # (b, ct) -> x tile [128, 1024] f32
        hnt = {}     # (b, ct) -> hn tile [128, 1024] bf16
        qkt = {}     # (b, j) -> j in 0..3: Q m-tiles 0,1; K m-tiles 2,3
        vtt = {}     # b -> v^T tile [128, 2048] bf16 (s-tile t at 256t, head h at +64h)
        expt = {}    # (pair, a) -> expS^T tile [128, 8192] bf16
        att = {}     # (b, ct) -> normalized attn out [128, 1024] bf16

        scratch = scpool.tile([128, 1024], F32, name="scr", tag="scr")

        def emit_gn(b):
            """GroupNorm stats + apply for batch b (DVE + tiny PE + ACT)."""
            stats = vecpool.tile([128, 4], F32, name=f"st{b}", tag="stats")
            veps = vecpool.tile([128, 2], F32, name=f"ve{b}", tag="veps")
            lnv = vecpool.tile([128, 2], F32, name=f"ln{b}", tag="lnv")
            rstd = vecpool.tile([128, 2], F32, name=f"rs{b}", tag="rstd")
            Av = vecpool.tile([128, 2], F32, name=f"A{b}", tag="Av")
            nBv = vecpool.tile([128, 2], F32, name=f"nB{b}", tag="nBv")
            for ct in range(CT):
                xtile = xpool.tile([128, 1024], F32, name=f"x{b}{ct}",
                                   tag=f"x{b}{ct}")
                nc.sync.dma_start(xtile[:], x_d[b, 128 * ct:128 * (ct + 1), :])
                xt[(b, ct)] = xtile
                nc.vector.tensor_reduce(
                    out=stats[:, 2 * ct:2 * ct + 1], in_=xtile[:],
                    axis=mybir.AxisListType.X, op=OP.add)
                nc.vector.scalar_tensor_tensor(
                    out=scratch[:], in0=xtile[:], scalar=1.0, in1=xtile[:],
                    op0=OP.bypass, op1=OP.mult,
                    accum_out=stats[:, 2 * ct + 1:2 * ct + 2])
            for ct in range(CT):
                # group-average via G matmul: gps = [mean_g, E2_g] replicated
                gps = ps_qkv.tile([128, 2], F32, name=f"g{b}{ct}", tag="qkv")
                nc.tensor.matmul(out=gps[:], lhsT=G[:],
                                 rhs=stats[:, 2 * ct:2 * ct + 2],
                                 start=True, stop=True)
                gsb = vecpool.tile([128, 2], F32, name=f"gs{b}{ct}",
                                   tag=f"gsb{ct}")
                nc.vector.tensor_copy(gsb[:], gps[:])
                # veps = mean^2 - E2  (so var = -veps)
                nc.vector.scalar_tensor_tensor(
                    out=veps[:, ct:ct + 1], in0=gsb[:, 0:1], scalar=gsb[:, 0:1],
                    in1=gsb[:, 1:2], op0=OP.mult, op1=OP.subtract)
                # rstd = exp(-0.5 * ln(var + eps))
                nc.scalar.activation(lnv[:, ct:ct + 1], veps[:, ct:ct + 1],
                                     AF.Ln, bias=epsb[:, 0:1], scale=-1.0)
                nc.scalar.activation(rstd[:, ct:ct + 1], lnv[:, ct:ct + 1],
                                     AF.Exp, scale=-0.5)
                # A = rstd * nw ; negB = mean*A - nb   (hn = x*A - negB)
                nc.vector.tensor_mul(Av[:, ct:ct + 1], rstd[:, ct:ct + 1],
                                     nw[:, ct:ct + 1])
                nc.vector.scalar_tensor_tensor(
                    out=nBv[:, ct:ct + 1], in0=gsb[:, 0:1],
                    scalar=Av[:, ct:ct + 1], in1=nb[:, ct:ct + 1],
                    op0=OP.mult, op1=OP.subtract)
                hn = hnpool.tile([128, 1024], BF16, name=f"hn{b}{ct}",
                                 tag=f"hn{b}{ct}")
                nc.vector.tensor_scalar(
                    out=hn[:], in0=xt[(b, ct)][:], scalar1=Av[:, ct:ct + 1],
                    scalar2=nBv[:, ct:ct + 1], op0=OP.mult, op1=OP.subtract)
                hnt[(b, ct)] = hn

        def emit_qkv(b):
            """Q,K as [o,s] tiles; V directly transposed as v^T [s, d] tiles."""
            for j in range(4):
                qk = qkpool.tile([128, 1024], BF16, name=f"qk{b}{j}",
                                 tag=f"qk{b}{j}")
                for qc in range(QC):
                    ps = ps_qkv.tile([128, 512], F32, name=f"qp{b}{j}{qc}",
                                     tag="qkv")
                    for k in range(CT):
                        nc.tensor.matmul(
                            out=ps[:],
                            lhsT=wq[k][:, 128 * j:128 * (j + 1)],
                            rhs=hnt[(b, k)][:, 512 * qc:512 * (qc + 1)],
                            start=(k == 0), stop=(k == CT - 1))
                    nc.vector.tensor_scalar(
                        out=qk[:, 512 * qc:512 * (qc + 1)], in0=ps[:],
                        scalar1=qkb[:, j:j + 1], scalar2=None, op0=OP.add)
                qkt[(b, j)] = qk
            # V^T: s-tiles, out [128 (s), 256 (dd)]
            vt = vtpool.tile([128, 2048], BF16, name=f"vt{b}", tag=f"vt{b}")
            for t in range(KT):
                ps = ps_qkv.tile([128, 256], F32, name=f"vp{b}{t}", tag="qkv")
                for k in range(CT):
                    nc.tensor.matmul(
                        out=ps[:],
                        lhsT=hnt[(b, k)][:, 128 * t:128 * (t + 1)],
                        rhs=wq[k][:, 512:768],
                        start=(k == 0), stop=(k == CT - 1))
                nc.vector.scalar_tensor_tensor(
                    out=vt[:, 256 * t:256 * (t + 1)], in0=ps[:], scalar=1.0,
                    in1=bv[:], op0=OP.bypass, op1=OP.add)
            vtt[b] = vt

        def emit_scores(p):
            """mm1 + exp for pair p: batch p//2, heads (0,1) or (2,3)."""
            b, hp = divmod(p, 2)
            qA = qkt[(b, hp)]      # Q m-tile hp: head 2hp rows 0-63, 2hp+1 rows 64-127
            kA = qkt[(b, 2 + hp)]  # K m-tile
            eA = expool.tile([128, 8192], BF16, name=f"ex{p}a", tag=f"ex{p % 2}a")
            eB = expool.tile([128, 8192], BF16, name=f"ex{p}b", tag=f"ex{p % 2}b")
            expt[(p, 0)], expt[(p, 1)] = eA, eB
            for t in range(KT):
                chA = ps_scores.tile([128, 1024], F32, name=f"sA{p}{t}", tag="sc")
                chB = ps_scores.tile([128, 1024], F32, name=f"sB{p}{t}", tag="sc")
                for qc in range(QC):
                    nc.tensor.matmul(
                        out=chA[:, 512 * qc:512 * (qc + 1)],
                        lhsT=kA[0:64, 128 * t:128 * (t + 1)],
                        rhs=qA[0:64, 512 * qc:512 * (qc + 1)],
                        start=True, stop=True, tile_position=(0, 0))
                    nc.tensor.matmul(
                        out=chB[:, 512 * qc:512 * (qc + 1)],
                        lhsT=kA[64:128, 128 * t:128 * (t + 1)],
                        rhs=qA[64:128, 512 * qc:512 * (qc + 1)],
                        start=True, stop=True, tile_position=(64, 0))
                nc.scalar.activation(eA[:, 1024 * t:1024 * (t + 1)], chA[:],
                                     AF.Exp, scale=0.125)
                nc.scalar.activation(eB[:, 1024 * t:1024 * (t + 1)], chB[:],
                                     AF.Exp, scale=0.125)

        def emit_mm2den(p):
            """P@V (col-tiled head pair) + denominators + normalize."""
            b, hp = divmod(p, 2)
            eA, eB = expt[(p, 0)], expt[(p, 1)]
            vt = vtt[b]
            hA, hB = 2 * hp, 2 * hp + 1
            u = ps_attn.tile([128, 1024], F32, name=f"u{p}", tag="at")
            for qc in range(QC):
                for t in range(KT):
                    nc.tensor.matmul(
                        out=u[0:64, 512 * qc:512 * (qc + 1)],
                        lhsT=vt[:, 256 * t + 64 * hA:256 * t + 64 * hA + 64],
                        rhs=eA[:, 1024 * t + 512 * qc:1024 * t + 512 * (qc + 1)],
                        start=(t == 0), stop=(t == KT - 1),
                        tile_position=(0, 0), skip_group_check=True)
                    nc.tensor.matmul(
                        out=u[64:128, 512 * qc:512 * (qc + 1)],
                        lhsT=vt[:, 256 * t + 64 * hB:256 * t + 64 * hB + 64],
                        rhs=eB[:, 1024 * t + 512 * qc:1024 * t + 512 * (qc + 1)],
                        start=(t == 0), stop=(t == KT - 1),
                        tile_position=(0, 64), skip_group_check=True)
            # denominators into two 1-bank tiles from the qkv pool so the den
            # matmuls never wait on (or hold) the `u` slot; denw is all-ones
            # over its 64 columns, so each den tile holds the denominator
            # replicated across partitions 0-63 / 64-127.
            rc = bcpool.tile([128, 1024], F32, name=f"rc{p}", tag="rc")
            for qc in range(QC):
                den = ps_qkv.tile([128, 512], F32, name=f"dn{p}{qc}", tag="qkv")
                for t in range(KT):
                    nc.tensor.matmul(
                        out=den[0:64, :],
                        lhsT=denw[:],
                        rhs=eA[:, 1024 * t + 512 * qc:1024 * t + 512 * (qc + 1)],
                        start=(t == 0), stop=(t == KT - 1),
                        tile_position=(0, 0), skip_group_check=True)
                    nc.tensor.matmul(
                        out=den[64:128, :],
                        lhsT=denw[:],
                        rhs=eB[:, 1024 * t + 512 * qc:1024 * t + 512 * (qc + 1)],
                        start=(t == 0), stop=(t == KT - 1),
                        tile_position=(0, 64), skip_group_check=True)
                nc.vector.reciprocal(rc[:, 512 * qc:512 * (qc + 1)], den[:])
            at = atpool.tile([128, 1024], BF16, name=f"at{p}", tag=f"at{p % 2}")
            nc.vector.tensor_mul(at[:], u[:], rc[:])
            att[(b, hp)] = at

        def emit_proj(b):
            """proj + residual + bias, then store."""
            for m in range(CT):
                ps = ps_attn.tile([128, 1024], F32, name=f"pj{b}{m}", tag="at")
                for qc in range(QC):
                    for k in range(CT):
                        nc.tensor.matmul(
                            out=ps[:, 512 * qc:512 * (qc + 1)],
                            lhsT=wp[k][:, 128 * m:128 * (m + 1)],
                            rhs=att[(b, k)][:, 512 * qc:512 * (qc + 1)],
                            start=(k == 0), stop=(k == CT - 1))
                osb = opool.tile([128, 1024], F32, name=f"o{b}{m}", tag="osb")
                nc.vector.scalar_tensor_tensor(
                    out=osb[:], in0=ps[:], scalar=pb[:, m:m + 1],
                    in1=xt[(b, m)][:], op0=OP.add, op1=OP.add)
                nc.sync.dma_start(out_d[b, 128 * m:128 * (m + 1), :], osb[:])

        # ---- software-pipelined emission ----
        emit_gn(0)
        emit_qkv(0)
        emit_gn(1)
        emit_scores(0)
        emit_qkv(1)
        emit_scores(1)
        emit_mm2den(0)
        emit_scores(2)
        emit_mm2den(1)
        emit_proj(0)
        emit_scores(3)
        emit_mm2den(2)
        emit_mm2den(3)
        emit_proj(1)

    nc.compile()
    return nc


_NC = None


def _get_nc():
    global _NC
    if _NC is None:
        _NC = build_nc()
    return _NC


def make_in_maps(x, norm_w, norm_b, qkv_w, qkv_b, proj_w, proj_b):
    x = np.asarray(x, dtype=np.float32)
    B = x.shape[0]
    assert B == N_CORES * B_PER

    wqkvT = np.ascontiguousarray(np.asarray(qkv_w, np.float32).T).astype(
        ml_dtypes.bfloat16)  # [C, 3C]
    wprojT = np.ascontiguousarray(np.asarray(proj_w, np.float32).T).astype(
        ml_dtypes.bfloat16)
    qkb = np.ascontiguousarray(
        np.asarray(qkv_b[:512], np.float32).reshape(4, 128).T)  # [128, 4]
    bv = np.broadcast_to(np.asarray(qkv_b[512:768], np.float32),
                         (128, C)).copy()
    pb = np.ascontiguousarray(np.asarray(proj_b, np.float32).reshape(2, 128).T)
    nw = np.ascontiguousarray(np.asarray(norm_w, np.float32).reshape(2, 128).T)
    nb = np.ascontiguousarray(np.asarray(norm_b, np.float32).reshape(2, 128).T)
    # block-diagonal group-average matrix, 1/(32*1024) normalizer folded in
    G = np.zeros((128, 128), np.float32)
    for g in range(4):
        G[32 * g:32 * (g + 1), 32 * g:32 * (g + 1)] = 1.0 / (32.0 * 1024.0)
    denw = np.ones((128, 64), np.float32).astype(ml_dtypes.bfloat16)

    xs = x.reshape(N_CORES, B_PER, C, S)
    common = dict(wqkvT=wqkvT, wprojT=wprojT, qkb=qkb, bv=bv, pb=pb, nw=nw,
                  nb=nb, G=G, denw=denw)
    return [dict(x=np.ascontiguousarray(xs[i]), **common)
            for i in range(N_CORES)]


def kernel(x, norm_w, norm_b, qkv_w, qkv_b, proj_w, proj_b):
    in_maps = make_in_maps(x, norm_w, norm_b, qkv_w, qkv_b, proj_w, proj_b)
    nc = _get_nc()
    res = run_bass_kernel_spmd(nc, in_maps, core_ids=list(range(N_CORES)))
    out = np.stack([res.results[i]["out"] for i in range(N_CORES)], axis=0)
    return out.reshape(x.shape[0], C, 32, 32).astype(np.float32)

